# revision 16
# baseline (speedup 1.0000x reference)
"""Trainium2 Bass kernel for a 2-step BasicNCA2D cell update.

Strategy
--------
Data-parallel over batch: 8 images, one per NeuronCore. Per core the two NCA
steps are fused on-chip (x never round-trips to DRAM between steps).

Per step the math is
    y  = depthwise_conv5x5(x, conv_w) + conv_b        (reflect padding)
    h  = relu([x, y] @ fc0_w + fc0_b)
    dx = h @ fc1_w
    x' = concat([x[..., :1], x[..., 1:] + dx])

Split fc0 = [W1; W2] (x-path / y-path). Then
    h_pre = x @ W1  +  sum_{di,dj} x_shift(di,dj) @ M[di,dj]  + b_eff,
    M[di,dj] = diag(conv_w[di,dj]) @ W2,  b_eff = fc0_b + conv_b @ W2.

The 25-tap sum runs on the PE in fp8(e4m3) with perf_mode=DoubleRow: channels
are packed dense (24 -> 96 partitions for 4 rows), and the two vertical
half-stacks of each output group (v-blocks m and m+1, offset -2 rows) form the
DoubleRow pair, so conv+fc0's y-part for 4 rows x 512 cols is 5 double-rate
matmuls (one per horizontal shift dj), half the cost of 10 fp32r matmuls.
fp8 blocks live in one big "ring" tile [96, 129, 528]; the DoubleRow moving
operand is the 3D AP ring[:, m:m+2, dj:dj+512] (pair stride = 528 B). Conv
weights are pre-scaled by S=32 to stay in e4m3's normal range; the relu
activation rescales with scale=1/S.

The precision-critical x @ W1 term stays exact: state tiles are group-ALIGNED
(rows 4m..4m+3 at partitions f*24+c), so x@W1 is ONE fp32r matmul per group
into the same PSUM accumulation. For step 2, x2 = x1 + dx1 is never
materialized in fp32; instead x2@W1 = x1@W1 + h1@V with V = fc1_w @ W1[1:]
(one more fp32r matmul), and the fp8 ring for step 2 is written directly by
DVE adds (psum dx1 + x1 -> fp8). fc1 accumulates dx1+dx2 in a PSUM bank held
across both steps, so the residual is a single add at the end:
out = x1 + (dx1 + dx2). Measured relative error ~6e-3 (budget 2e-2).

DMAs are batched 4 groups per transfer to amortize HWDGE overhead. Cost-model
bottleneck is the PE at ~2.1us per 4-row group (13 matmuls) for both steps.
"""

import numpy as np
import ml_dtypes

import concourse.mybir as mybir
import concourse.tile as tile
from concourse import bacc
from concourse.bass_utils import run_bass_kernel_spmd

F32 = mybir.dt.float32
F32R = mybir.dt.float32r
FP8 = mybir.dt.float8e4
E4 = ml_dtypes.float8_e4m3

H = 512
W = 512
C = 24
CP = 32  # hidden padding per row-slot (psum layout), kept for compat
HD = 32
NCORES = 8
NG = H // 4          # 128 output groups of 4 rows
NB = NG + 1          # 129 ring v-blocks per step
PITCH = 528          # fp8 ring block pitch (>=516, %16==0, >=512B descriptors)
CS = 32              # channel slots per row (24 real + 8 zero)
P = 4 * CS           # 128 state partitions (row f, channel c) -> f*32+c
S = 32.0             # fp8 weight prescale
XB = 4               # groups per x/out DMA batch
DR = mybir.MatmulPerfMode.DoubleRow


def _build_nc(steps: int, repeat: int = 1):
    nc = bacc.Bacc("TRN2", target_bir_lowering=False, debug=False)

    XA = nc.dram_tensor("XA", [P, NG, W], F32R, kind="ExternalInput")
    X8 = nc.dram_tensor("X8", [P, NB, PITCH], FP8, kind="ExternalInput")
    WD = nc.dram_tensor("WD", [P, 5, 2, 128], FP8, kind="ExternalInput")
    W1S = nc.dram_tensor("W1S", [P, 128], F32R, kind="ExternalInput")
    VS = nc.dram_tensor("VS", [128, 128], F32R, kind="ExternalInput")
    WC = nc.dram_tensor("WC", [128, P], F32R, kind="ExternalInput")
    BIAS = nc.dram_tensor("BIAS", [128, 1], F32, kind="ExternalInput")
    Y = nc.dram_tensor("Y", [P, NG, W], F32, kind="ExternalOutput")

    two_steps = steps == 2
    assert steps in (1, 2)

    with tile.TileContext(nc) as tc:
        with (
            tc.tile_pool(name="wpool", bufs=1) as wpool,
            tc.tile_pool(name="rpool", bufs=1) as rpool,
            tc.tile_pool(name="xpool", bufs=4) as xpool,
            tc.tile_pool(name="hpool", bufs=2) as hpool,
            tc.tile_pool(name="opool", bufs=2) as opool,
            tc.tile_pool(name="pp1", bufs=2, space="PSUM") as pp1,
            tc.tile_pool(name="pp2", bufs=2, space="PSUM") as pp2,
            tc.tile_pool(name="ppdx", bufs=4, space="PSUM") as ppdx,
        ):
            # ---- weights ----
            wd_t = wpool.tile([P, 5, 2, 128], FP8, tag="wd")
            nc.sync.dma_start(wd_t[:], WD.ap())
            w1s_t = wpool.tile([P, 128], F32R, tag="w1s")
            nc.sync.dma_start(w1s_t[:], W1S.ap())
            wc_t = wpool.tile([128, P], F32R, tag="wc")
            nc.sync.dma_start(wc_t[:], WC.ap())
            bias_t = wpool.tile([128, 1], F32, tag="bias")
            nc.sync.dma_start(bias_t[:], BIAS.ap())
            if two_steps:
                vs_t = wpool.tile([128, 128], F32R, tag="vs")
                nc.sync.dma_start(vs_t[:], VS.ap())

            ring1 = rpool.tile([P, NB, PITCH], FP8, tag="ring1", name="ring1")
            ring2 = None
            if two_steps:
                ring2 = rpool.tile([P, NB, PITCH], FP8, tag="ring2", name="ring2")

            for _rep in range(repeat):
                xal = {}    # batch index -> [P, XB, 512] tile
                h1s = {}    # group -> h1 tile
                dxs = {}    # group -> held dx psum tile
                outt = {}   # batch index -> out tile

                def load(i):
                    b = i // XB
                    if i % XB or b > NG // XB:
                        return
                    if b < NG // XB:
                        t = xpool.tile([P, XB, W], F32R, tag="xa", name=f"xa{b}")
                        xal[b] = t
                        nc.sync.dma_start(
                            t[:], XA.ap()[:, XB * b : XB * (b + 1), :]
                        )
                        nc.sync.dma_start(
                            ring1[:, XB * b : XB * (b + 1), :],
                            X8.ap()[:, XB * b : XB * (b + 1), :],
                        )
                    else:
                        nc.sync.dma_start(
                            ring1[:, NB - 1 : NB, :], X8.ap()[:, NB - 1 : NB, :]
                        )

                def xa_of(m):
                    return xal[m // XB][:, m % XB, :]

                def halo_cols(ring, k):
                    for vc, pc in ((0, 4), (1, 3), (514, 512), (515, 511)):
                        nc.gpsimd.tensor_copy(
                            ring[:, k, vc : vc + 1], ring[:, k, pc : pc + 1]
                        )

                def conv_group(hp, ring, m):
                    for dj in range(5):
                        nc.tensor.matmul(
                            hp[:],
                            wd_t[:, dj],
                            ring[:, m : m + 2, dj : dj + W],
                            start=(dj == 0),
                            stop=False,
                            perf_mode=DR,
                        )

                def stage1(m):
                    xa = xa_of(m)
                    hp = pp1.tile([128, W], F32, tag="hp1", name=f"hp1_{m}")
                    conv_group(hp, ring1, m)
                    nc.tensor.matmul(hp[:], w1s_t[:], xa, start=False, stop=True)
                    h1 = hpool.tile([128, W], F32R, tag="h1", name=f"h1_{m}", bufs=4)
                    nc.scalar.activation(
                        h1[:], hp[:], mybir.ActivationFunctionType.Relu,
                        bias=bias_t[:], scale=1.0 / S,
                    )
                    dxp = ppdx.tile([P, W], F32, tag="dx", name=f"dx_{m}")
                    nc.tensor.matmul(
                        dxp[:], wc_t[:], h1[:], start=True, stop=not two_steps
                    )
                    if not two_steps:
                        finish(m, dxp)
                        return
                    h1s[m] = h1
                    dxs[m] = dxp
                    # fp8 ring for step 2: x2 = x1 + dx1, straight to e4m3
                    nc.vector.tensor_add(
                        ring2[64:128, m, 2:514], dxp[0:64, :].bitcast(F32R), xa[0:64, :]
                    )
                    nc.vector.tensor_add(
                        ring2[0:64, m + 1, 2:514],
                        dxp[64:128, :].bitcast(F32R),
                        xa[64:128, :],
                    )
                    if m == 0:
                        # top reflect rows: row -1 := row 1, row -2 := row 2
                        nc.gpsimd.tensor_copy(
                            ring2[32:64, 0, 2:514], ring2[96:128, 0, 2:514]
                        )
                        nc.gpsimd.tensor_copy(
                            ring2[0:32, 0, 2:514], ring2[0:32, 1, 2:514]
                        )
                    halo_cols(ring2, m)
                    if m == NG - 1:
                        # bottom block: row 512 := row 510, row 513 := row 509
                        nc.gpsimd.tensor_copy(
                            ring2[64:96, NB - 1, 2:514], ring2[0:32, NB - 1, 2:514]
                        )
                        nc.gpsimd.tensor_copy(
                            ring2[96:128, NB - 1, 2:514], ring2[96:128, NB - 2, 2:514]
                        )
                        halo_cols(ring2, NB - 1)

                def finish(m, dxp):
                    b, q = m // XB, m % XB
                    if q == 0:
                        outt[b] = opool.tile([P, XB, W], F32, tag="out", name=f"o{b}")
                    nc.vector.tensor_add(
                        outt[b][:, q, :], dxp[:].bitcast(F32R), xa_of(m)
                    )
                    if q == XB - 1:
                        nc.sync.dma_start(
                            Y.ap()[:, XB * b : XB * (b + 1), :], outt.pop(b)[:]
                        )

                def stage2(m):
                    xa = xa_of(m)
                    hp = pp2.tile([128, W], F32, tag="hp2", name=f"hp2_{m}")
                    conv_group(hp, ring2, m)
                    nc.tensor.matmul(hp[:], w1s_t[:], xa, start=False, stop=False)
                    nc.tensor.matmul(
                        hp[:], vs_t[:], h1s.pop(m)[:], start=False, stop=True
                    )
                    h2 = hpool.tile([128, W], F32R, tag="h2", name=f"h2_{m}", bufs=2)
                    nc.scalar.activation(
                        h2[:], hp[:], mybir.ActivationFunctionType.Relu,
                        bias=bias_t[:], scale=1.0 / S,
                    )
                    dxp = dxs.pop(m)
                    nc.tensor.matmul(dxp[:], wc_t[:], h2[:], start=False, stop=True)
                    finish(m, dxp)

                s2_lag = 2 if two_steps else 0
                for i in range(NG + 2 + s2_lag + 1):
                    load(i)
                    m1 = i - 2
                    if 0 <= m1 < NG:
                        stage1(m1)
                    if two_steps:
                        m2 = i - 2 - s2_lag
                        if 0 <= m2 < NG:
                            stage2(m2)

    nc.compile()
    return nc


_NC_CACHE = {}
_REPEAT = 1


def _get_nc(steps):
    key = (steps, _REPEAT)
    if key not in _NC_CACHE:
        _NC_CACHE[key] = _build_nc(steps, repeat=_REPEAT)
    return _NC_CACHE[key]


def _prep_weights(conv_w, conv_b, fc0_w, fc0_b, fc1_w):
    cw = np.asarray(conv_w, np.float64)[:, :, 0, :]      # [5,5,24]
    W1 = np.asarray(fc0_w, np.float64)[:C]               # [24,32]
    W2 = np.asarray(fc0_w, np.float64)[C:]               # [24,32]
    fc1 = np.asarray(fc1_w, np.float64)                  # [32,23]

    M = cw[:, :, :, None] * W2[None, None, :, :]         # [5,5,24,32] (ki,kj,c,h)

    WDh = np.zeros((P, 5, 2, 128), np.float32)
    for g in range(4):
        for f in range(4):
            if 0 <= g - f <= 4:   # block A: di = g - f - 2
                WDh[g * CS : g * CS + C, :, 0, f * HD : f * HD + HD] = (
                    M[g - f].transpose(1, 0, 2) * S
                )
            if 0 <= g + 4 - f <= 4:  # block B: di = g + 2 - f
                WDh[g * CS : g * CS + C, :, 1, f * HD : f * HD + HD] = (
                    M[g + 4 - f].transpose(1, 0, 2) * S
                )
    WDh = np.clip(WDh, -240, 240).astype(E4)

    W1Sh = np.zeros((P, 128), np.float32)
    VSh = np.zeros((128, 128), np.float32)
    WCh = np.zeros((128, P), np.float32)
    Vm = fc1 @ W1[1:C]                                   # [32,32]
    for f in range(4):
        W1Sh[f * CS : f * CS + C, f * HD : f * HD + HD] = W1 * S
        VSh[f * HD : f * HD + HD, f * HD : f * HD + HD] = Vm * S
        WCh[f * HD : f * HD + HD, f * CS + 1 : f * CS + C] = fc1
    bias_eff = (np.asarray(fc0_b, np.float64) + np.asarray(conv_b, np.float64) @ W2)
    BIASh = np.tile(bias_eff.astype(np.float32), 4).reshape(128, 1)
    return WDh, W1Sh.astype(np.float32), VSh.astype(np.float32), WCh.astype(
        np.float32
    ), BIASh


def _prep_x8(x_chw):
    """[24,512,512] f32 -> fp8 ring [96, 129, 528] with reflect halos."""
    pad = np.pad(x_chw, ((0, 0), (2, 2), (2, 2)), mode="reflect")  # [24,516,516]
    r = pad.reshape(C, NB, 4, H + 4).transpose(2, 0, 1, 3)  # [4,24,NB,516]
    out = np.zeros((4, CS, NB, PITCH), E4)
    out[:, :C, :, : H + 4] = np.clip(r, -240, 240).astype(E4)
    return out.reshape(P, NB, PITCH)


def _prep_xa(x_chw):
    """[24,512,512] f32 -> aligned state [128, 128, 512]: (f*32+c, m, w)."""
    r = x_chw.reshape(C, NG, 4, W).transpose(2, 0, 1, 3)  # [4,24,NG,512]
    out = np.zeros((4, CS, NG, W), np.float32)
    out[:, :C] = r
    return out.reshape(P, NG, W)


def _run_pass(x_chw, wts, steps):
    """One device invocation: `steps` NCA steps on x [B, C, H, W] fp32."""
    B = x_chw.shape[0]
    WDh, W1Sh, VSh, WCh, BIASh = wts
    nc = _get_nc(steps)
    in_maps = []
    for i in range(NCORES):
        xi = x_chw[i % B]
        in_maps.append(
            {
                "XA": _prep_xa(xi),
                "X8": _prep_x8(xi),
                "WD": WDh,
                "W1S": W1Sh,
                "VS": VSh,
                "WC": WCh,
                "BIAS": BIASh,
            }
        )
    res = run_bass_kernel_spmd(nc, in_maps, core_ids=list(range(NCORES)))
    globals()["LAST_RESULTS"] = res
    out = []
    for i in range(B):
        yi = res.results[i]["Y"]  # [128, 128, 512]
        out.append(
            yi.reshape(4, CS, NG, W)[:, :C]
            .transpose(1, 2, 0, 3)
            .reshape(C, H, W)
        )
    return np.stack(out)  # [B, C, H, W]


def kernel(x, conv_w, conv_b, fc0_w, fc0_b, fc1_w, steps):
    steps = int(steps)
    x = np.asarray(x, np.float32)
    B = x.shape[0]
    assert x.shape == (B, H, W, C) and 1 <= B <= NCORES, x.shape
    if steps <= 0:
        return x.copy()

    wts = _prep_weights(conv_w, conv_b, fc0_w, fc0_b, fc1_w)
    x_chw = np.ascontiguousarray(x.transpose(0, 3, 1, 2))
    # device pipeline supports 2 fused steps; decompose larger step counts
    while steps > 0:
        n = 2 if steps >= 2 else 1
        x_chw = _run_pass(x_chw, wts, n)
        steps -= n
    return np.ascontiguousarray(x_chw.transpose(0, 2, 3, 1)).astype(np.float32)


if __name__ == "__main__":
    rng = np.random.default_rng(0)
    inputs = {
        "x": rng.standard_normal((8, H, W, C), dtype=np.float32),
        "conv_w": (rng.standard_normal((5, 5, 1, C)) * 0.1).astype(np.float32),
        "conv_b": (rng.standard_normal((C,)) * 0.1).astype(np.float32),
        "fc0_w": (rng.standard_normal((2 * C, HD)) * 0.1).astype(np.float32),
        "fc0_b": (rng.standard_normal((HD,)) * 0.1).astype(np.float32),
        "fc1_w": (rng.standard_normal((HD, C - 1)) * 0.1).astype(np.float32),
        "steps": 2,
    }
    out = kernel(**inputs)
    print(out.shape, out.dtype)


# revision 17
# speedup vs baseline: 1.0331x; 1.0331x over previous
"""Trainium2 Bass kernel for a 2-step BasicNCA2D cell update.

Strategy
--------
Data-parallel over batch: 8 images, one per NeuronCore. Per core the two NCA
steps are fused on-chip (x never round-trips to DRAM between steps).

Per step the math is
    y  = depthwise_conv5x5(x, conv_w) + conv_b        (reflect padding)
    h  = relu([x, y] @ fc0_w + fc0_b)
    dx = h @ fc1_w
    x' = concat([x[..., :1], x[..., 1:] + dx])

Split fc0 = [W1; W2] (x-path / y-path). Then
    h_pre = x @ W1  +  sum_{di,dj} x_shift(di,dj) @ M[di,dj]  + b_eff,
    M[di,dj] = diag(conv_w[di,dj]) @ W2,  b_eff = fc0_b + conv_b @ W2.

The 25-tap sum runs on the PE in fp8(e4m3) with perf_mode=DoubleRow: channels
are packed dense (24 -> 96 partitions for 4 rows), and the two vertical
half-stacks of each output group (v-blocks m and m+1, offset -2 rows) form the
DoubleRow pair, so conv+fc0's y-part for 4 rows x 512 cols is 5 double-rate
matmuls (one per horizontal shift dj), half the cost of 10 fp32r matmuls.
fp8 blocks live in one big "ring" tile [96, 129, 528]; the DoubleRow moving
operand is the 3D AP ring[:, m:m+2, dj:dj+512] (pair stride = 528 B). Conv
weights are pre-scaled by S=32 to stay in e4m3's normal range; the relu
activation rescales with scale=1/S.

The precision-critical x @ W1 term stays exact: state tiles are group-ALIGNED
(rows 4m..4m+3 at partitions f*24+c), so x@W1 is ONE fp32r matmul per group
into the same PSUM accumulation. For step 2, x2 = x1 + dx1 is never
materialized in fp32; instead x2@W1 = x1@W1 + h1@V with V = fc1_w @ W1[1:]
(one more fp32r matmul), and the fp8 ring for step 2 is written directly by
DVE adds (psum dx1 + x1 -> fp8). fc1 accumulates dx1+dx2 in a PSUM bank held
across both steps, so the residual is a single add at the end:
out = x1 + (dx1 + dx2). Measured relative error ~6e-3 (budget 2e-2).

DMAs are batched 4 groups per transfer to amortize HWDGE overhead. Cost-model
bottleneck is the PE at ~2.1us per 4-row group (13 matmuls) for both steps.
"""

import numpy as np
import ml_dtypes

import concourse.mybir as mybir
import concourse.tile as tile
from concourse import bacc
from concourse.bass_utils import run_bass_kernel_spmd

F32 = mybir.dt.float32
F32R = mybir.dt.float32r
FP8 = mybir.dt.float8e4
E4 = ml_dtypes.float8_e4m3

H = 512
W = 512
C = 24
CP = 32  # hidden padding per row-slot (psum layout), kept for compat
HD = 32
NCORES = 8
NG = H // 4          # 128 output groups of 4 rows
NB = NG + 1          # 129 ring v-blocks per step
PITCH = 528          # fp8 ring block pitch (>=516, %16==0, >=512B descriptors)
CS = 32              # channel slots per row (24 real + 8 zero)
P = 4 * CS           # 128 state partitions (row f, channel c) -> f*32+c
S = 32.0             # fp8 weight prescale
XB = 4               # groups per x/out DMA batch
DR = mybir.MatmulPerfMode.DoubleRow


def _build_nc(steps: int, repeat: int = 1):
    nc = bacc.Bacc("TRN2", target_bir_lowering=False, debug=False)

    XA = nc.dram_tensor("XA", [P, NG, W], F32R, kind="ExternalInput")
    X8 = nc.dram_tensor("X8", [P, NB, PITCH], FP8, kind="ExternalInput")
    WD = nc.dram_tensor("WD", [P, 5, 2, 128], FP8, kind="ExternalInput")
    W1S = nc.dram_tensor("W1S", [P, 128], F32R, kind="ExternalInput")
    VS = nc.dram_tensor("VS", [128, 128], F32R, kind="ExternalInput")
    WC = nc.dram_tensor("WC", [128, P], F32R, kind="ExternalInput")
    BIAS = nc.dram_tensor("BIAS", [128, 1], F32, kind="ExternalInput")
    Y = nc.dram_tensor("Y", [P, NG, W], F32, kind="ExternalOutput")

    two_steps = steps == 2
    assert steps in (1, 2)

    with tile.TileContext(nc) as tc:
        with (
            tc.tile_pool(name="wpool", bufs=1) as wpool,
            tc.tile_pool(name="rpool", bufs=1) as rpool,
            tc.tile_pool(name="xpool", bufs=4) as xpool,
            tc.tile_pool(name="hpool", bufs=2) as hpool,
            tc.tile_pool(name="opool", bufs=2) as opool,
            tc.tile_pool(name="pp1", bufs=2, space="PSUM") as pp1,
            tc.tile_pool(name="pp2", bufs=2, space="PSUM") as pp2,
            tc.tile_pool(name="ppdx", bufs=4, space="PSUM") as ppdx,
        ):
            # ---- weights ----
            wd_t = wpool.tile([P, 5, 2, 128], FP8, tag="wd")
            nc.sync.dma_start(wd_t[:], WD.ap())
            w1s_t = wpool.tile([P, 128], F32R, tag="w1s")
            nc.sync.dma_start(w1s_t[:], W1S.ap())
            wc_t = wpool.tile([128, P], F32R, tag="wc")
            nc.sync.dma_start(wc_t[:], WC.ap())
            bias_t = wpool.tile([128, 1], F32, tag="bias")
            nc.sync.dma_start(bias_t[:], BIAS.ap())
            if two_steps:
                vs_t = wpool.tile([128, 128], F32R, tag="vs")
                nc.sync.dma_start(vs_t[:], VS.ap())

            ring1 = rpool.tile([P, NB, PITCH], FP8, tag="ring1", name="ring1")
            ring2 = None
            if two_steps:
                ring2 = rpool.tile([P, NB, PITCH], FP8, tag="ring2", name="ring2")

            for _rep in range(repeat):
                xal = {}    # batch index -> [P, XB, 512] tile
                h1s = {}    # group -> h1 tile
                h2s = {}    # group -> h2 tile
                dxs = {}    # group -> held dx psum tile
                outt = {}   # batch index -> out tile

                def load_batch(b):
                    if b < NG // XB:
                        t = xpool.tile([P, XB, W], F32R, tag="xa", name=f"xa{b}")
                        xal[b] = t
                        nc.sync.dma_start(
                            t[:], XA.ap()[:, XB * b : XB * (b + 1), :]
                        )
                        nc.sync.dma_start(
                            ring1[:, XB * b : XB * (b + 1), :],
                            X8.ap()[:, XB * b : XB * (b + 1), :],
                        )
                    elif b == NG // XB:
                        nc.sync.dma_start(
                            ring1[:, NB - 1 : NB, :], X8.ap()[:, NB - 1 : NB, :]
                        )

                def load(i):
                    if i == 0:
                        load_batch(0)
                    if i % XB == 0:
                        load_batch(i // XB + 1)

                def xa_of(m):
                    return xal[m // XB][:, m % XB, :]

                def halo_cols(ring, k):
                    for vc, pc in ((0, 4), (1, 3), (514, 512), (515, 511)):
                        nc.gpsimd.tensor_copy(
                            ring[:, k, vc : vc + 1], ring[:, k, pc : pc + 1]
                        )

                def conv_group(hp, ring, m):
                    for dj in range(5):
                        nc.tensor.matmul(
                            hp[:],
                            wd_t[:, dj],
                            ring[:, m : m + 2, dj : dj + W],
                            start=(dj == 0),
                            stop=False,
                            perf_mode=DR,
                        )

                def stage1a(m):
                    hp = pp1.tile([128, W], F32, tag="hp1", name=f"hp1_{m}")
                    conv_group(hp, ring1, m)
                    nc.tensor.matmul(hp[:], w1s_t[:], xa_of(m), start=False, stop=True)
                    h1 = hpool.tile([128, W], F32R, tag="h1", name=f"h1_{m}", bufs=4)
                    nc.scalar.activation(
                        h1[:], hp[:], mybir.ActivationFunctionType.Relu,
                        bias=bias_t[:], scale=1.0 / S,
                    )
                    h1s[m] = h1

                def stage1b(m):
                    xa = xa_of(m)
                    dxp = ppdx.tile([P, W], F32, tag="dx", name=f"dx_{m}")
                    nc.tensor.matmul(
                        dxp[:], wc_t[:], h1s[m][:], start=True, stop=not two_steps
                    )
                    if not two_steps:
                        h1s.pop(m)
                        finish(m, dxp)
                        return
                    dxs[m] = dxp
                    # fp8 ring for step 2: x2 = x1 + dx1, straight to e4m3
                    nc.vector.tensor_add(
                        ring2[64:128, m, 2:514], dxp[0:64, :].bitcast(F32R), xa[0:64, :]
                    )
                    nc.vector.tensor_add(
                        ring2[0:64, m + 1, 2:514],
                        dxp[64:128, :].bitcast(F32R),
                        xa[64:128, :],
                    )
                    if m == 0:
                        # top reflect rows: row -1 := row 1, row -2 := row 2
                        nc.gpsimd.tensor_copy(
                            ring2[32:64, 0, 2:514], ring2[96:128, 0, 2:514]
                        )
                        nc.gpsimd.tensor_copy(
                            ring2[0:32, 0, 2:514], ring2[0:32, 1, 2:514]
                        )
                    halo_cols(ring2, m)
                    if m == NG - 1:
                        # bottom block: row 512 := row 510, row 513 := row 509
                        nc.gpsimd.tensor_copy(
                            ring2[64:96, NB - 1, 2:514], ring2[0:32, NB - 1, 2:514]
                        )
                        nc.gpsimd.tensor_copy(
                            ring2[96:128, NB - 1, 2:514], ring2[96:128, NB - 2, 2:514]
                        )
                        halo_cols(ring2, NB - 1)

                def finish(m, dxp):
                    b, q = m // XB, m % XB
                    if q == 0:
                        outt[b] = opool.tile([P, XB, W], F32, tag="out", name=f"o{b}")
                    nc.vector.tensor_add(
                        outt[b][:, q, :], dxp[:].bitcast(F32R), xa_of(m)
                    )
                    if q == XB - 1:
                        # store on the Activation HWDGE queue so pending
                        # stores never head-of-line-block input loads (SP)
                        nc.scalar.dma_start(
                            Y.ap()[:, XB * b : XB * (b + 1), :], outt.pop(b)[:]
                        )

                def stage2a(m):
                    hp = pp2.tile([128, W], F32, tag="hp2", name=f"hp2_{m}")
                    conv_group(hp, ring2, m)
                    nc.tensor.matmul(hp[:], w1s_t[:], xa_of(m), start=False, stop=False)
                    nc.tensor.matmul(
                        hp[:], vs_t[:], h1s.pop(m)[:], start=False, stop=True
                    )
                    h2 = hpool.tile([128, W], F32R, tag="h2", name=f"h2_{m}", bufs=2)
                    nc.scalar.activation(
                        h2[:], hp[:], mybir.ActivationFunctionType.Relu,
                        bias=bias_t[:], scale=1.0 / S,
                    )
                    h2s[m] = h2

                def stage2b(m):
                    dxp = dxs.pop(m)
                    nc.tensor.matmul(
                        dxp[:], wc_t[:], h2s.pop(m)[:], start=False, stop=True
                    )
                    finish(m, dxp)

                if two_steps:
                    phases = ((2, stage1a), (3, stage1b), (5, stage2a), (6, stage2b))
                else:
                    phases = ((2, stage1a), (3, stage1b))
                last_lag = phases[-1][0]
                for i in range(NG + last_lag + 1):
                    load(i)
                    for lag, fn in phases:
                        m = i - lag
                        if 0 <= m < NG:
                            fn(m)

    nc.compile()
    return nc


_NC_CACHE = {}
_REPEAT = 1


def _get_nc(steps):
    key = (steps, _REPEAT)
    if key not in _NC_CACHE:
        _NC_CACHE[key] = _build_nc(steps, repeat=_REPEAT)
    return _NC_CACHE[key]


def _prep_weights(conv_w, conv_b, fc0_w, fc0_b, fc1_w):
    cw = np.asarray(conv_w, np.float64)[:, :, 0, :]      # [5,5,24]
    W1 = np.asarray(fc0_w, np.float64)[:C]               # [24,32]
    W2 = np.asarray(fc0_w, np.float64)[C:]               # [24,32]
    fc1 = np.asarray(fc1_w, np.float64)                  # [32,23]

    M = cw[:, :, :, None] * W2[None, None, :, :]         # [5,5,24,32] (ki,kj,c,h)

    WDh = np.zeros((P, 5, 2, 128), np.float32)
    for g in range(4):
        for f in range(4):
            if 0 <= g - f <= 4:   # block A: di = g - f - 2
                WDh[g * CS : g * CS + C, :, 0, f * HD : f * HD + HD] = (
                    M[g - f].transpose(1, 0, 2) * S
                )
            if 0 <= g + 4 - f <= 4:  # block B: di = g + 2 - f
                WDh[g * CS : g * CS + C, :, 1, f * HD : f * HD + HD] = (
                    M[g + 4 - f].transpose(1, 0, 2) * S
                )
    WDh = np.clip(WDh, -240, 240).astype(E4)

    W1Sh = np.zeros((P, 128), np.float32)
    VSh = np.zeros((128, 128), np.float32)
    WCh = np.zeros((128, P), np.float32)
    Vm = fc1 @ W1[1:C]                                   # [32,32]
    for f in range(4):
        W1Sh[f * CS : f * CS + C, f * HD : f * HD + HD] = W1 * S
        VSh[f * HD : f * HD + HD, f * HD : f * HD + HD] = Vm * S
        WCh[f * HD : f * HD + HD, f * CS + 1 : f * CS + C] = fc1
    bias_eff = (np.asarray(fc0_b, np.float64) + np.asarray(conv_b, np.float64) @ W2)
    BIASh = np.tile(bias_eff.astype(np.float32), 4).reshape(128, 1)
    return WDh, W1Sh.astype(np.float32), VSh.astype(np.float32), WCh.astype(
        np.float32
    ), BIASh


def _prep_x8(x_chw):
    """[24,512,512] f32 -> fp8 ring [96, 129, 528] with reflect halos."""
    pad = np.pad(x_chw, ((0, 0), (2, 2), (2, 2)), mode="reflect")  # [24,516,516]
    r = pad.reshape(C, NB, 4, H + 4).transpose(2, 0, 1, 3)  # [4,24,NB,516]
    out = np.zeros((4, CS, NB, PITCH), E4)
    out[:, :C, :, : H + 4] = np.clip(r, -240, 240).astype(E4)
    return out.reshape(P, NB, PITCH)


def _prep_xa(x_chw):
    """[24,512,512] f32 -> aligned state [128, 128, 512]: (f*32+c, m, w)."""
    r = x_chw.reshape(C, NG, 4, W).transpose(2, 0, 1, 3)  # [4,24,NG,512]
    out = np.zeros((4, CS, NG, W), np.float32)
    out[:, :C] = r
    return out.reshape(P, NG, W)


def _run_pass(x_chw, wts, steps):
    """One device invocation: `steps` NCA steps on x [B, C, H, W] fp32."""
    B = x_chw.shape[0]
    WDh, W1Sh, VSh, WCh, BIASh = wts
    nc = _get_nc(steps)
    in_maps = []
    for i in range(NCORES):
        xi = x_chw[i % B]
        in_maps.append(
            {
                "XA": _prep_xa(xi),
                "X8": _prep_x8(xi),
                "WD": WDh,
                "W1S": W1Sh,
                "VS": VSh,
                "WC": WCh,
                "BIAS": BIASh,
            }
        )
    res = run_bass_kernel_spmd(nc, in_maps, core_ids=list(range(NCORES)))
    globals()["LAST_RESULTS"] = res
    out = []
    for i in range(B):
        yi = res.results[i]["Y"]  # [128, 128, 512]
        out.append(
            yi.reshape(4, CS, NG, W)[:, :C]
            .transpose(1, 2, 0, 3)
            .reshape(C, H, W)
        )
    return np.stack(out)  # [B, C, H, W]


def kernel(x, conv_w, conv_b, fc0_w, fc0_b, fc1_w, steps):
    steps = int(steps)
    x = np.asarray(x, np.float32)
    B = x.shape[0]
    assert x.shape == (B, H, W, C) and 1 <= B <= NCORES, x.shape
    if steps <= 0:
        return x.copy()

    wts = _prep_weights(conv_w, conv_b, fc0_w, fc0_b, fc1_w)
    x_chw = np.ascontiguousarray(x.transpose(0, 3, 1, 2))
    # device pipeline supports 2 fused steps; decompose larger step counts
    while steps > 0:
        n = 2 if steps >= 2 else 1
        x_chw = _run_pass(x_chw, wts, n)
        steps -= n
    return np.ascontiguousarray(x_chw.transpose(0, 2, 3, 1)).astype(np.float32)


if __name__ == "__main__":
    rng = np.random.default_rng(0)
    inputs = {
        "x": rng.standard_normal((8, H, W, C), dtype=np.float32),
        "conv_w": (rng.standard_normal((5, 5, 1, C)) * 0.1).astype(np.float32),
        "conv_b": (rng.standard_normal((C,)) * 0.1).astype(np.float32),
        "fc0_w": (rng.standard_normal((2 * C, HD)) * 0.1).astype(np.float32),
        "fc0_b": (rng.standard_normal((HD,)) * 0.1).astype(np.float32),
        "fc1_w": (rng.standard_normal((HD, C - 1)) * 0.1).astype(np.float32),
        "steps": 2,
    }
    out = kernel(**inputs)
    print(out.shape, out.dtype)
